# revision 5
# baseline (speedup 1.0000x reference)
"""Trainium2 Bass kernel for nn_CrosslayerDecoder.

Reference computation:
    out[:, l, :] = sum_{i<=l} features[:, i, :] @ W_l[i]  + b[l]
with B=64, L=12, DF=4096, DA=768 (fp32).

The work is 78 independent [64,4096]@[4096,768] products (one per (l, i)
pair), each weight block read exactly once -> memory-bound on the ~981 MB
of weights.  Sharding: the 78 (l, i) pairs are split across the 8 cores
(10/10/10/10/10/10/9+pad/9+pad), so every per-core weight DMA is a large
fully-contiguous block.  Each core computes partial outputs [pairs, B, DA];
the host sums partials into layers and adds the bias.

Numerics: fp32 inputs are split on the host into bf16 hi + bf16 lo halves
(same total bytes over the wire).  Each k-tile contributes three bf16
matmuls (hi*hi, hi*lo, lo*hi) accumulated into the same fp32 PSUM group,
giving ~1e-5 relative error at full PE rate (plain fp32 matmul is
quarter-rate on the PE; float32r is full-rate but only tf32 precision).

All DRAM operands are pre-packed on the host into the exact SBUF tile
layout, so every DMA is a single fully-contiguous block transfer.
"""

import numpy as np
import ml_dtypes

import concourse.bass as bass  # noqa: F401
import concourse.mybir as mybir
import concourse.tile as tile
from concourse import bacc
from concourse.bass_utils import run_bass_kernel_spmd

B, L, DF, DA = 64, 12, 4096, 768
NCORES = 8
PAIRS_PER_CORE = 10
P = 128                  # SBUF partitions
KT = DF // P             # 32 k-tiles per pair
CH = 2                   # weight chunks per pair (per hi/lo stream)
KS = KT // CH            # 16 k-tiles per chunk (3.15 MB per DMA)
NH = DA // 2             # 384 = PSUM tile free dim (<=512 fp32 per bank)

BF16 = ml_dtypes.bfloat16

# i-major pair order: all (l, i) with l >= i, i ascending.  Consecutive
# runs go to consecutive cores; cores 6,7 have 9 real pairs + 1 zero pad.
_PAIRS = [(l, i) for i in range(L) for l in range(i, L)]
_COUNTS = [10, 10, 10, 10, 10, 10, 9, 9]
_ASSIGN: list[list[tuple[int, int] | None]] = []
_off = 0
for _c in _COUNTS:
    sl: list[tuple[int, int] | None] = list(_PAIRS[_off : _off + _c])
    sl += [None] * (PAIRS_PER_CORE - _c)
    _ASSIGN.append(sl)
    _off += _c
assert _off == len(_PAIRS) == 78

_NC_CACHE = None


def _build_program():
    """One SPMD Bass program (identical on all 8 cores)."""
    global _NC_CACHE
    if _NC_CACHE is not None:
        return _NC_CACHE

    dt = mybir.dt.bfloat16
    nc = bacc.Bacc("TRN2", target_bir_lowering=False, debug=False)
    fh_in = nc.dram_tensor(
        "f_hi", [PAIRS_PER_CORE, P, KT * B], dt, kind="ExternalInput"
    ).ap()
    fl_in = nc.dram_tensor(
        "f_lo", [PAIRS_PER_CORE, P, KT * B], dt, kind="ExternalInput"
    ).ap()
    wh_in = nc.dram_tensor(
        "w_hi", [PAIRS_PER_CORE, CH, P, KS * DA], dt, kind="ExternalInput"
    ).ap()
    wl_in = nc.dram_tensor(
        "w_lo", [PAIRS_PER_CORE, CH, P, KS * DA], dt, kind="ExternalInput"
    ).ap()
    o_out = nc.dram_tensor(
        "out", [PAIRS_PER_CORE, B, DA], mybir.dt.float32, kind="ExternalOutput"
    ).ap()

    with tile.TileContext(nc) as tc:
        with (
            tc.tile_pool(name="f", bufs=2) as fpool,
            tc.tile_pool(name="w", bufs=3) as wpool,
            tc.tile_pool(name="ps", bufs=2, space="PSUM") as pspool,
            tc.tile_pool(name="o", bufs=2) as opool,
        ):
            for p in range(PAIRS_PER_CORE):
                fh = fpool.tile([P, KT * B], dt, tag="fh")
                fl = fpool.tile([P, KT * B], dt, tag="fl")
                # features + outputs ride the SWDGE (gpsimd) path, keeping
                # both HWDGE rings (sync, scalar) exclusively for the two
                # weight streams so per-DMA setup gaps overlap.
                nc.gpsimd.dma_start(out=fh[:], in_=fh_in[p])
                nc.gpsimd.dma_start(out=fl[:], in_=fl_in[p])
                ps_a = pspool.tile([B, NH], mybir.dt.float32)
                ps_b = pspool.tile([B, NH], mybir.dt.float32)
                for c in range(CH):
                    wh = wpool.tile([P, KS * DA], dt, tag="wh")
                    wl = wpool.tile([P, KS * DA], dt, tag="wl")
                    nc.sync.dma_start(out=wh[:], in_=wh_in[p, c])
                    nc.scalar.dma_start(out=wl[:], in_=wl_in[p, c])
                    for s in range(KS):
                        k = c * KS + s
                        lh = fh[:, k * B : (k + 1) * B]
                        ll = fl[:, k * B : (k + 1) * B]
                        whA = wh[:, s * DA : s * DA + NH]
                        whB = wh[:, s * DA + NH : (s + 1) * DA]
                        wlA = wl[:, s * DA : s * DA + NH]
                        wlB = wl[:, s * DA + NH : (s + 1) * DA]
                        first = k == 0
                        last = k == KT - 1
                        # hi*hi, hi*lo share the stationary lhsT tile
                        nc.tensor.matmul(ps_a[:], lhsT=lh, rhs=whA, start=first, stop=False)
                        nc.tensor.matmul(ps_b[:], lhsT=lh, rhs=whB, start=first, stop=False)
                        nc.tensor.matmul(ps_a[:], lhsT=lh, rhs=wlA, start=False, stop=False)
                        nc.tensor.matmul(ps_b[:], lhsT=lh, rhs=wlB, start=False, stop=False)
                        nc.tensor.matmul(ps_a[:], lhsT=ll, rhs=whA, start=False, stop=last)
                        nc.tensor.matmul(ps_b[:], lhsT=ll, rhs=whB, start=False, stop=last)
                ot = opool.tile([B, DA], mybir.dt.float32)
                nc.vector.tensor_copy(ot[:, :NH], ps_a[:])
                nc.vector.tensor_copy(ot[:, NH:], ps_b[:])
                nc.gpsimd.dma_start(out=o_out[p], in_=ot[:])

    nc.compile()
    _NC_CACHE = nc
    return nc


def _split_bf16(x32):
    hi = x32.astype(BF16)
    lo = (x32 - hi.astype(np.float32)).astype(BF16)
    return hi, lo


def _pack_w(w32):
    """[DF, DA] fp32 -> hi/lo bf16 in SBUF chunk layout [CH, P, KS*DA]."""
    hi, lo = _split_bf16(w32)

    def pack(x):
        return np.ascontiguousarray(
            x.reshape(CH, KS, P, DA).transpose(0, 2, 1, 3).reshape(CH, P, KS * DA)
        )

    return pack(hi), pack(lo)


def _prep_inputs(features, Ws):
    """Per-core in_maps: pre-tiled bf16 hi/lo feature + weight blocks."""
    features = np.ascontiguousarray(np.asarray(features, dtype=np.float32))
    # featT tile for feature index i: [DF, B] -> [P, KT*B] with layout
    # [partition, (ktile, batch)] so the SBUF tile is one contiguous DMA.
    fh_tiles, fl_tiles = {}, {}
    for i in range(L):
        x = features[:, i, :]                          # [B, DF]
        t = x.T.reshape(KT, P, B).transpose(1, 0, 2)   # [P, KT, B]
        hi, lo = _split_bf16(np.ascontiguousarray(t.reshape(P, KT * B)))
        fh_tiles[i], fl_tiles[i] = hi, lo

    in_maps = []
    for core in range(NCORES):
        fh = np.zeros((PAIRS_PER_CORE, P, KT * B), dtype=BF16)
        fl = np.zeros((PAIRS_PER_CORE, P, KT * B), dtype=BF16)
        wh = np.zeros((PAIRS_PER_CORE, CH, P, KS * DA), dtype=BF16)
        wl = np.zeros((PAIRS_PER_CORE, CH, P, KS * DA), dtype=BF16)
        for slot, pair in enumerate(_ASSIGN[core]):
            if pair is None:
                continue
            l, i = pair
            fh[slot], fl[slot] = fh_tiles[i], fl_tiles[i]
            wh[slot], wl[slot] = _pack_w(np.asarray(Ws[l][i], dtype=np.float32))
        in_maps.append({"f_hi": fh, "f_lo": fl, "w_hi": wh, "w_lo": wl})
    return in_maps


def _assemble(results, b):
    out = np.zeros((B, L, DA), dtype=np.float32)
    for core in range(NCORES):
        o = np.asarray(results[core]["out"], dtype=np.float32)
        for slot, pair in enumerate(_ASSIGN[core]):
            if pair is None:
                continue
            l, _i = pair
            out[:, l, :] += o[slot]
    out += np.asarray(b, dtype=np.float32)[None, :, :]
    return out


def run(inputs: dict, trace: bool = False, **spmd_kwargs):
    """Compile (cached), run on 8 cores, return (full_output, BassKernelResults)."""
    Ws = [np.asarray(inputs[f"W_{l}"], dtype=np.float32) for l in range(L)]
    in_maps = _prep_inputs(inputs["features"], Ws)
    nc = _build_program()
    res = run_bass_kernel_spmd(
        nc, in_maps, list(range(NCORES)), trace=trace, **spmd_kwargs
    )
    out = _assemble(res.results, inputs["b"])
    return out, res


def kernel(**inputs) -> np.ndarray:
    out, _ = run(inputs)
    return out


# revision 7
# speedup vs baseline: 1.0186x; 1.0186x over previous
"""Trainium2 Bass kernel for nn_CrosslayerDecoder.

Reference computation:
    out[:, l, :] = sum_{i<=l} features[:, i, :] @ W_l[i]  + b[l]
with B=64, L=12, DF=4096, DA=768 (fp32).

The work is 78 independent [64,4096]@[4096,768] products (one per (l, i)
pair), each weight block read exactly once -> memory-bound on the ~981 MB
of weights.  Sharding: the 78 (l, i) pairs are split across the 8 cores
(10/10/10/10/10/10/9+pad/9+pad), so every per-core weight DMA is a large
fully-contiguous block.  Each core computes partial outputs [pairs, B, DA];
the host sums partials into layers and adds the bias.

Numerics: fp32 inputs are split on the host into bf16 hi + bf16 lo halves
(same total bytes over the wire).  Each k-tile contributes three bf16
matmuls (hi*hi, hi*lo, lo*hi) accumulated into the same fp32 PSUM group,
giving ~1e-5 relative error at full PE rate (plain fp32 matmul is
quarter-rate on the PE; float32r is full-rate but only tf32 precision).

All DRAM operands are pre-packed on the host into the exact SBUF tile
layout, so every DMA is a single fully-contiguous block transfer.
"""

import numpy as np
import ml_dtypes

import concourse.bass as bass  # noqa: F401
import concourse.mybir as mybir
import concourse.tile as tile
from concourse import bacc
from concourse.bass_utils import run_bass_kernel_spmd

B, L, DF, DA = 64, 12, 4096, 768
NCORES = 8
PAIRS_PER_CORE = 10
P = 128                  # SBUF partitions
KT = DF // P             # 32 k-tiles per pair
CH = 2                   # weight chunks per pair (per hi/lo stream)
KS = KT // CH            # 16 k-tiles per chunk (3.15 MB per DMA)
NH = DA // 2             # 384 = PSUM tile free dim (<=512 fp32 per bank)

BF16 = ml_dtypes.bfloat16

# i-major pair order: all (l, i) with l >= i, i ascending.  Consecutive
# runs go to consecutive cores; cores 6,7 have 9 real pairs + 1 zero pad.
_PAIRS = [(l, i) for i in range(L) for l in range(i, L)]
_COUNTS = [10, 10, 10, 10, 10, 10, 9, 9]
_ASSIGN: list[list[tuple[int, int] | None]] = []
_off = 0
for _c in _COUNTS:
    sl: list[tuple[int, int] | None] = list(_PAIRS[_off : _off + _c])
    sl += [None] * (PAIRS_PER_CORE - _c)
    _ASSIGN.append(sl)
    _off += _c
assert _off == len(_PAIRS) == 78

_NC_CACHE = None


def _build_program():
    """One SPMD Bass program (identical on all 8 cores)."""
    global _NC_CACHE
    if _NC_CACHE is not None:
        return _NC_CACHE

    dt = mybir.dt.bfloat16
    nc = bacc.Bacc("TRN2", target_bir_lowering=False, debug=False)
    fh_in = nc.dram_tensor(
        "f_hi", [PAIRS_PER_CORE, P, KT * B], dt, kind="ExternalInput"
    ).ap()
    fl_in = nc.dram_tensor(
        "f_lo", [PAIRS_PER_CORE, P, KT * B], dt, kind="ExternalInput"
    ).ap()
    wh_in = nc.dram_tensor(
        "w_hi", [PAIRS_PER_CORE, CH, P, KS * DA], dt, kind="ExternalInput"
    ).ap()
    wl_in = nc.dram_tensor(
        "w_lo", [PAIRS_PER_CORE, CH, P, KS * DA], dt, kind="ExternalInput"
    ).ap()
    o_out = nc.dram_tensor(
        "out", [PAIRS_PER_CORE, B, DA], mybir.dt.float32, kind="ExternalOutput"
    ).ap()

    with tile.TileContext(nc) as tc:
        with (
            tc.tile_pool(name="f", bufs=2) as fpool,
            tc.tile_pool(name="w", bufs=3) as wpool,
            tc.tile_pool(name="ps", bufs=2, space="PSUM") as pspool,
            tc.tile_pool(name="o", bufs=2) as opool,
        ):
            for p in range(PAIRS_PER_CORE):
                fh = fpool.tile([P, KT * B], dt, tag="fh")
                fl = fpool.tile([P, KT * B], dt, tag="fl")
                # Split all traffic across the two HWDGE rings (sync=SP,
                # scalar=ACT) so consecutive DMAs' setup/completion overlap.
                nc.sync.dma_start(out=fh[:], in_=fh_in[p])
                nc.scalar.dma_start(out=fl[:], in_=fl_in[p])
                ps_a = pspool.tile([B, NH], mybir.dt.float32)
                ps_b = pspool.tile([B, NH], mybir.dt.float32)
                for c in range(CH):
                    wh = wpool.tile([P, KS * DA], dt, tag="wh")
                    wl = wpool.tile([P, KS * DA], dt, tag="wl")
                    # alternate rings per chunk to keep both rings balanced
                    ring_a = nc.sync if c % 2 == 0 else nc.scalar
                    ring_b = nc.scalar if c % 2 == 0 else nc.sync
                    ring_a.dma_start(out=wh[:], in_=wh_in[p, c])
                    ring_b.dma_start(out=wl[:], in_=wl_in[p, c])
                    for s in range(KS):
                        k = c * KS + s
                        lh = fh[:, k * B : (k + 1) * B]
                        ll = fl[:, k * B : (k + 1) * B]
                        whA = wh[:, s * DA : s * DA + NH]
                        whB = wh[:, s * DA + NH : (s + 1) * DA]
                        wlA = wl[:, s * DA : s * DA + NH]
                        wlB = wl[:, s * DA + NH : (s + 1) * DA]
                        first = k == 0
                        last = k == KT - 1
                        # hi*hi, hi*lo share the stationary lhsT tile
                        nc.tensor.matmul(ps_a[:], lhsT=lh, rhs=whA, start=first, stop=False)
                        nc.tensor.matmul(ps_b[:], lhsT=lh, rhs=whB, start=first, stop=False)
                        nc.tensor.matmul(ps_a[:], lhsT=lh, rhs=wlA, start=False, stop=False)
                        nc.tensor.matmul(ps_b[:], lhsT=lh, rhs=wlB, start=False, stop=False)
                        nc.tensor.matmul(ps_a[:], lhsT=ll, rhs=whA, start=False, stop=last)
                        nc.tensor.matmul(ps_b[:], lhsT=ll, rhs=whB, start=False, stop=last)
                ot = opool.tile([B, DA], mybir.dt.float32)
                nc.vector.tensor_copy(ot[:, :NH], ps_a[:])
                nc.vector.tensor_copy(ot[:, NH:], ps_b[:])
                (nc.sync if p % 2 == 0 else nc.scalar).dma_start(
                    out=o_out[p], in_=ot[:]
                )

    nc.compile()
    _NC_CACHE = nc
    return nc


def _split_bf16(x32):
    hi = x32.astype(BF16)
    lo = (x32 - hi.astype(np.float32)).astype(BF16)
    return hi, lo


def _pack_w(w32):
    """[DF, DA] fp32 -> hi/lo bf16 in SBUF chunk layout [CH, P, KS*DA]."""
    hi, lo = _split_bf16(w32)

    def pack(x):
        return np.ascontiguousarray(
            x.reshape(CH, KS, P, DA).transpose(0, 2, 1, 3).reshape(CH, P, KS * DA)
        )

    return pack(hi), pack(lo)


def _prep_inputs(features, Ws):
    """Per-core in_maps: pre-tiled bf16 hi/lo feature + weight blocks."""
    features = np.ascontiguousarray(np.asarray(features, dtype=np.float32))
    # featT tile for feature index i: [DF, B] -> [P, KT*B] with layout
    # [partition, (ktile, batch)] so the SBUF tile is one contiguous DMA.
    fh_tiles, fl_tiles = {}, {}
    for i in range(L):
        x = features[:, i, :]                          # [B, DF]
        t = x.T.reshape(KT, P, B).transpose(1, 0, 2)   # [P, KT, B]
        hi, lo = _split_bf16(np.ascontiguousarray(t.reshape(P, KT * B)))
        fh_tiles[i], fl_tiles[i] = hi, lo

    in_maps = []
    for core in range(NCORES):
        fh = np.zeros((PAIRS_PER_CORE, P, KT * B), dtype=BF16)
        fl = np.zeros((PAIRS_PER_CORE, P, KT * B), dtype=BF16)
        wh = np.zeros((PAIRS_PER_CORE, CH, P, KS * DA), dtype=BF16)
        wl = np.zeros((PAIRS_PER_CORE, CH, P, KS * DA), dtype=BF16)
        for slot, pair in enumerate(_ASSIGN[core]):
            if pair is None:
                continue
            l, i = pair
            fh[slot], fl[slot] = fh_tiles[i], fl_tiles[i]
            wh[slot], wl[slot] = _pack_w(np.asarray(Ws[l][i], dtype=np.float32))
        in_maps.append({"f_hi": fh, "f_lo": fl, "w_hi": wh, "w_lo": wl})
    return in_maps


def _assemble(results, b):
    out = np.zeros((B, L, DA), dtype=np.float32)
    for core in range(NCORES):
        o = np.asarray(results[core]["out"], dtype=np.float32)
        for slot, pair in enumerate(_ASSIGN[core]):
            if pair is None:
                continue
            l, _i = pair
            out[:, l, :] += o[slot]
    out += np.asarray(b, dtype=np.float32)[None, :, :]
    return out


def run(inputs: dict, trace: bool = False, **spmd_kwargs):
    """Compile (cached), run on 8 cores, return (full_output, BassKernelResults)."""
    Ws = [np.asarray(inputs[f"W_{l}"], dtype=np.float32) for l in range(L)]
    in_maps = _prep_inputs(inputs["features"], Ws)
    nc = _build_program()
    res = run_bass_kernel_spmd(
        nc, in_maps, list(range(NCORES)), trace=trace, **spmd_kwargs
    )
    out = _assemble(res.results, inputs["b"])
    return out, res


def kernel(**inputs) -> np.ndarray:
    out, _ = run(inputs)
    return out


# revision 8
# speedup vs baseline: 1.2857x; 1.2622x over previous
"""Trainium2 Bass kernel for nn_CrosslayerDecoder.

Reference computation:
    out[:, l, :] = sum_{i<=l} features[:, i, :] @ W_l[i]  + b[l]
with B=64, L=12, DF=4096, DA=768 (fp32).

The work is 78 independent [64,4096]@[4096,768] products (one per (l, i)
pair), each weight block read exactly once -> memory-bound on the ~981 MB
of weights.  Sharding: the 78 (l, i) pairs are split across the 8 cores
(10/10/10/10/10/10/9+pad/9+pad), so every per-core weight DMA is a large
fully-contiguous block.  Each core computes partial outputs [pairs, B, DA];
the host sums partials into layers and adds the bias.

Numerics: fp32 inputs are split on the host into bf16 hi + bf16 lo halves
(same total bytes over the wire).  Each k-tile contributes three bf16
matmuls (hi*hi, hi*lo, lo*hi) accumulated into the same fp32 PSUM group,
giving ~1e-5 relative error at full PE rate (plain fp32 matmul is
quarter-rate on the PE; float32r is full-rate but only tf32 precision).

All DRAM operands are pre-packed on the host into the exact SBUF tile
layout, so every DMA is a single fully-contiguous block transfer.
"""

import numpy as np
import ml_dtypes

import concourse.bass as bass  # noqa: F401
import concourse.mybir as mybir
import concourse.tile as tile
from concourse import bacc
from concourse.bass_utils import run_bass_kernel_spmd

B, L, DF, DA = 64, 12, 4096, 768
NCORES = 8
PAIRS_PER_CORE = 10
P = 128                  # SBUF partitions
KT = DF // P             # 32 k-tiles per pair
CH = 4                   # weight chunks per pair (per hi/lo stream)
KS = KT // CH            # 8 k-tiles per chunk (1.57 MB per DMA)
NH = DA // 2             # 384 = PSUM tile free dim (<=512 fp32 per bank)

BF16 = ml_dtypes.bfloat16

# i-major pair order: all (l, i) with l >= i, i ascending.  Consecutive
# runs go to consecutive cores; cores 6,7 have 9 real pairs + 1 zero pad.
_PAIRS = [(l, i) for i in range(L) for l in range(i, L)]
_COUNTS = [10, 10, 10, 10, 10, 10, 9, 9]
_ASSIGN: list[list[tuple[int, int] | None]] = []
_off = 0
for _c in _COUNTS:
    sl: list[tuple[int, int] | None] = list(_PAIRS[_off : _off + _c])
    sl += [None] * (PAIRS_PER_CORE - _c)
    _ASSIGN.append(sl)
    _off += _c
assert _off == len(_PAIRS) == 78

_NC_CACHE = None


def _build_program():
    """One SPMD Bass program (identical on all 8 cores)."""
    global _NC_CACHE
    if _NC_CACHE is not None:
        return _NC_CACHE

    dt = mybir.dt.bfloat16
    nc = bacc.Bacc("TRN2", target_bir_lowering=False, debug=False)
    fh_in = nc.dram_tensor(
        "f_hi", [PAIRS_PER_CORE, P, KT * B], dt, kind="ExternalInput"
    ).ap()
    fl_in = nc.dram_tensor(
        "f_lo", [PAIRS_PER_CORE, P, KT * B], dt, kind="ExternalInput"
    ).ap()
    wh_in = nc.dram_tensor(
        "w_hi", [PAIRS_PER_CORE, CH, P, KS * DA], dt, kind="ExternalInput"
    ).ap()
    wl_in = nc.dram_tensor(
        "w_lo", [PAIRS_PER_CORE, CH, P, KS * DA], dt, kind="ExternalInput"
    ).ap()
    o_out = nc.dram_tensor(
        "out", [PAIRS_PER_CORE, B, DA], mybir.dt.float32, kind="ExternalOutput"
    ).ap()

    with tile.TileContext(nc) as tc:
        with (
            tc.tile_pool(name="f", bufs=2) as fpool,
            tc.tile_pool(name="w", bufs=5) as wpool,
            tc.tile_pool(name="ps", bufs=2, space="PSUM") as pspool,
            tc.tile_pool(name="o", bufs=2) as opool,
        ):
            for p in range(PAIRS_PER_CORE):
                fh = fpool.tile([P, KT * B], dt, tag="fh")
                fl = fpool.tile([P, KT * B], dt, tag="fl")
                # Split all traffic across the two HWDGE rings (sync=SP,
                # scalar=ACT) so consecutive DMAs' setup/completion overlap.
                nc.sync.dma_start(out=fh[:], in_=fh_in[p])
                nc.scalar.dma_start(out=fl[:], in_=fl_in[p])
                ps_a = pspool.tile([B, NH], mybir.dt.float32)
                ps_b = pspool.tile([B, NH], mybir.dt.float32)
                for c in range(CH):
                    wh = wpool.tile([P, KS * DA], dt, tag="wh")
                    wl = wpool.tile([P, KS * DA], dt, tag="wl")
                    # alternate rings per chunk to keep both rings balanced
                    ring_a = nc.sync if c % 2 == 0 else nc.scalar
                    ring_b = nc.scalar if c % 2 == 0 else nc.sync
                    ring_a.dma_start(out=wh[:], in_=wh_in[p, c])
                    ring_b.dma_start(out=wl[:], in_=wl_in[p, c])
                    for s in range(KS):
                        k = c * KS + s
                        lh = fh[:, k * B : (k + 1) * B]
                        ll = fl[:, k * B : (k + 1) * B]
                        whA = wh[:, s * DA : s * DA + NH]
                        whB = wh[:, s * DA + NH : (s + 1) * DA]
                        wlA = wl[:, s * DA : s * DA + NH]
                        wlB = wl[:, s * DA + NH : (s + 1) * DA]
                        first = k == 0
                        last = k == KT - 1
                        # hi*hi, hi*lo share the stationary lhsT tile
                        nc.tensor.matmul(ps_a[:], lhsT=lh, rhs=whA, start=first, stop=False)
                        nc.tensor.matmul(ps_b[:], lhsT=lh, rhs=whB, start=first, stop=False)
                        nc.tensor.matmul(ps_a[:], lhsT=lh, rhs=wlA, start=False, stop=False)
                        nc.tensor.matmul(ps_b[:], lhsT=lh, rhs=wlB, start=False, stop=False)
                        nc.tensor.matmul(ps_a[:], lhsT=ll, rhs=whA, start=False, stop=last)
                        nc.tensor.matmul(ps_b[:], lhsT=ll, rhs=whB, start=False, stop=last)
                ot = opool.tile([B, DA], mybir.dt.float32)
                nc.vector.tensor_copy(ot[:, :NH], ps_a[:])
                nc.vector.tensor_copy(ot[:, NH:], ps_b[:])
                (nc.sync if p % 2 == 0 else nc.scalar).dma_start(
                    out=o_out[p], in_=ot[:]
                )

    nc.compile()
    _NC_CACHE = nc
    return nc


def _split_bf16(x32):
    hi = x32.astype(BF16)
    lo = (x32 - hi.astype(np.float32)).astype(BF16)
    return hi, lo


def _pack_w(w32):
    """[DF, DA] fp32 -> hi/lo bf16 in SBUF chunk layout [CH, P, KS*DA]."""
    hi, lo = _split_bf16(w32)

    def pack(x):
        return np.ascontiguousarray(
            x.reshape(CH, KS, P, DA).transpose(0, 2, 1, 3).reshape(CH, P, KS * DA)
        )

    return pack(hi), pack(lo)


def _prep_inputs(features, Ws):
    """Per-core in_maps: pre-tiled bf16 hi/lo feature + weight blocks."""
    features = np.ascontiguousarray(np.asarray(features, dtype=np.float32))
    # featT tile for feature index i: [DF, B] -> [P, KT*B] with layout
    # [partition, (ktile, batch)] so the SBUF tile is one contiguous DMA.
    fh_tiles, fl_tiles = {}, {}
    for i in range(L):
        x = features[:, i, :]                          # [B, DF]
        t = x.T.reshape(KT, P, B).transpose(1, 0, 2)   # [P, KT, B]
        hi, lo = _split_bf16(np.ascontiguousarray(t.reshape(P, KT * B)))
        fh_tiles[i], fl_tiles[i] = hi, lo

    in_maps = []
    for core in range(NCORES):
        fh = np.zeros((PAIRS_PER_CORE, P, KT * B), dtype=BF16)
        fl = np.zeros((PAIRS_PER_CORE, P, KT * B), dtype=BF16)
        wh = np.zeros((PAIRS_PER_CORE, CH, P, KS * DA), dtype=BF16)
        wl = np.zeros((PAIRS_PER_CORE, CH, P, KS * DA), dtype=BF16)
        for slot, pair in enumerate(_ASSIGN[core]):
            if pair is None:
                continue
            l, i = pair
            fh[slot], fl[slot] = fh_tiles[i], fl_tiles[i]
            wh[slot], wl[slot] = _pack_w(np.asarray(Ws[l][i], dtype=np.float32))
        in_maps.append({"f_hi": fh, "f_lo": fl, "w_hi": wh, "w_lo": wl})
    return in_maps


def _assemble(results, b):
    out = np.zeros((B, L, DA), dtype=np.float32)
    for core in range(NCORES):
        o = np.asarray(results[core]["out"], dtype=np.float32)
        for slot, pair in enumerate(_ASSIGN[core]):
            if pair is None:
                continue
            l, _i = pair
            out[:, l, :] += o[slot]
    out += np.asarray(b, dtype=np.float32)[None, :, :]
    return out


def run(inputs: dict, trace: bool = False, **spmd_kwargs):
    """Compile (cached), run on 8 cores, return (full_output, BassKernelResults)."""
    Ws = [np.asarray(inputs[f"W_{l}"], dtype=np.float32) for l in range(L)]
    in_maps = _prep_inputs(inputs["features"], Ws)
    nc = _build_program()
    res = run_bass_kernel_spmd(
        nc, in_maps, list(range(NCORES)), trace=trace, **spmd_kwargs
    )
    out = _assemble(res.results, inputs["b"])
    return out, res


def kernel(**inputs) -> np.ndarray:
    out, _ = run(inputs)
    return out


# revision 9
# speedup vs baseline: 1.3128x; 1.0211x over previous
"""Trainium2 Bass kernel for nn_CrosslayerDecoder.

Reference computation:
    out[:, l, :] = sum_{i<=l} features[:, i, :] @ W_l[i]  + b[l]
with B=64, L=12, DF=4096, DA=768 (fp32).

The work is 78 independent [64,4096]@[4096,768] products (one per (l, i)
pair), each weight block read exactly once -> memory-bound on the ~981 MB
of weights.  Eight specialized 1-core programs run concurrently, one per
NeuronCore.

Global work = 78 pairs x 4 chunks = 312 weight chunks (8 k-tiles each).
Each core gets exactly 39 consecutive chunks (perfect byte balance, no
padding).  A pair whose chunks span a core boundary is split by k-range;
each core emits one partial output per pair-segment and the host sums
segments into layers.  Distinct feature tiles are loaded once per core and
stay resident in SBUF (dedup vs the SPMD version's per-pair reloads).
"""

import numpy as np
import ml_dtypes

import concourse.mybir as mybir
import concourse.tile as tile
from concourse import bacc

B, L, DF, DA = 64, 12, 4096, 768
NCORES = 8
P = 128
KT = DF // P             # 32 k-tiles per pair
KS = 8                   # k-tiles per chunk
CPP = KT // KS           # 4 chunks per pair
NH = DA // 2             # 384

BF16 = ml_dtypes.bfloat16

_PAIRS = [(l, i) for i in range(L) for l in range(i, L)]
assert len(_PAIRS) == 78

# global chunk list: (pair_idx, chunk_in_pair)
_CHUNKS = [(pi, c) for pi in range(len(_PAIRS)) for c in range(CPP)]
assert len(_CHUNKS) == 312 and 312 % NCORES == 0
_PER = 312 // NCORES     # 39 chunks per core


def _core_plan(core):
    """Segments for one core: (l, i, islot, chunk_lo, chunk_hi) per segment.

    chunk range is within the pair (0..CPP); islot indexes this core's
    distinct-feature table.
    """
    chunks = _CHUNKS[core * _PER : (core + 1) * _PER]
    segs = []
    for pi, c in chunks:
        if segs and segs[-1][0] == pi and segs[-1][2] == c:
            segs[-1][2] += 1
        else:
            segs.append([pi, c, c + 1])
    plan = []
    islots = {}
    for pi, c0, c1 in segs:
        l, i = _PAIRS[pi]
        if i not in islots:
            islots[i] = len(islots)
        plan.append((l, i, islots[i], c0, c1))
    return plan, sorted(islots, key=islots.get)


_PLANS = [_core_plan(c) for c in range(NCORES)]
_NC_CACHE = [None] * NCORES


def _build_program(core):
    if _NC_CACHE[core] is not None:
        return _NC_CACHE[core]
    plan, i_list = _PLANS[core]
    n_seg = len(plan)
    n_islot = len(i_list)

    dt = mybir.dt.bfloat16
    nc = bacc.Bacc("TRN2", target_bir_lowering=False, debug=False)
    fh_in = nc.dram_tensor("f_hi", [n_islot, P, KT * B], dt, kind="ExternalInput").ap()
    fl_in = nc.dram_tensor("f_lo", [n_islot, P, KT * B], dt, kind="ExternalInput").ap()
    wh_in = nc.dram_tensor("w_hi", [_PER, P, KS * DA], dt, kind="ExternalInput").ap()
    wl_in = nc.dram_tensor("w_lo", [_PER, P, KS * DA], dt, kind="ExternalInput").ap()
    o_out = nc.dram_tensor(
        "out", [n_seg, B, DA], mybir.dt.float32, kind="ExternalOutput"
    ).ap()

    with tile.TileContext(nc) as tc:
        with (
            tc.tile_pool(name="f", bufs=1) as fpool,
            tc.tile_pool(name="w", bufs=5) as wpool,
            tc.tile_pool(name="ps", bufs=2, space="PSUM") as pspool,
            tc.tile_pool(name="o", bufs=2) as opool,
        ):
            # resident feature tiles, loaded once each
            fh_t, fl_t = [], []
            for j in range(n_islot):
                fh = fpool.tile([P, KT * B], dt, tag=f"fh{j}")
                fl = fpool.tile([P, KT * B], dt, tag=f"fl{j}")
                nc.sync.dma_start(out=fh[:], in_=fh_in[j])
                nc.scalar.dma_start(out=fl[:], in_=fl_in[j])
                fh_t.append(fh)
                fl_t.append(fl)

            gchunk = 0  # running index into this core's 39 weight chunks
            for seg_idx, (l, i, islot, c0, c1) in enumerate(plan):
                ps_a = pspool.tile([B, NH], mybir.dt.float32)
                ps_b = pspool.tile([B, NH], mybir.dt.float32)
                nchunks = c1 - c0
                for cc in range(nchunks):
                    wh = wpool.tile([P, KS * DA], dt, tag="wh")
                    wl = wpool.tile([P, KS * DA], dt, tag="wl")
                    ring_a = nc.sync if gchunk % 2 == 0 else nc.scalar
                    ring_b = nc.scalar if gchunk % 2 == 0 else nc.sync
                    ring_a.dma_start(out=wh[:], in_=wh_in[gchunk])
                    ring_b.dma_start(out=wl[:], in_=wl_in[gchunk])
                    for s in range(KS):
                        k = (c0 + cc) * KS + s       # k-tile within the pair
                        lh = fh_t[islot][:, k * B : (k + 1) * B]
                        ll = fl_t[islot][:, k * B : (k + 1) * B]
                        whA = wh[:, s * DA : s * DA + NH]
                        whB = wh[:, s * DA + NH : (s + 1) * DA]
                        wlA = wl[:, s * DA : s * DA + NH]
                        wlB = wl[:, s * DA + NH : (s + 1) * DA]
                        first = cc == 0 and s == 0
                        last = cc == nchunks - 1 and s == KS - 1
                        nc.tensor.matmul(ps_a[:], lhsT=lh, rhs=whA, start=first, stop=False)
                        nc.tensor.matmul(ps_b[:], lhsT=lh, rhs=whB, start=first, stop=False)
                        nc.tensor.matmul(ps_a[:], lhsT=lh, rhs=wlA, start=False, stop=False)
                        nc.tensor.matmul(ps_b[:], lhsT=lh, rhs=wlB, start=False, stop=False)
                        nc.tensor.matmul(ps_a[:], lhsT=ll, rhs=whA, start=False, stop=last)
                        nc.tensor.matmul(ps_b[:], lhsT=ll, rhs=whB, start=False, stop=last)
                    gchunk += 1
                ot = opool.tile([B, DA], mybir.dt.float32)
                nc.vector.tensor_copy(ot[:, :NH], ps_a[:])
                nc.vector.tensor_copy(ot[:, NH:], ps_b[:])
                (nc.sync if seg_idx % 2 == 0 else nc.scalar).dma_start(
                    out=o_out[seg_idx], in_=ot[:]
                )
    nc.compile()
    _NC_CACHE[core] = nc
    return nc


def _split_bf16(x32):
    hi = x32.astype(BF16)
    lo = (x32 - hi.astype(np.float32)).astype(BF16)
    return hi, lo


def _prep_inputs(features, Ws):
    features = np.ascontiguousarray(np.asarray(features, dtype=np.float32))
    fh_tiles, fl_tiles = {}, {}
    for i in range(L):
        x = features[:, i, :]
        t = x.T.reshape(KT, P, B).transpose(1, 0, 2)
        hi, lo = _split_bf16(np.ascontiguousarray(t.reshape(P, KT * B)))
        fh_tiles[i], fl_tiles[i] = hi, lo

    # per-pair packed weight chunks [CPP, P, KS*DA] hi/lo, built lazily
    packed = {}

    def pair_chunks(pi):
        if pi not in packed:
            l, i = _PAIRS[pi]
            w32 = np.asarray(Ws[l][i], dtype=np.float32)
            hi, lo = _split_bf16(w32)

            def pack(x):
                return np.ascontiguousarray(
                    x.reshape(CPP, KS, P, DA).transpose(0, 2, 1, 3).reshape(CPP, P, KS * DA)
                )

            packed[pi] = (pack(hi), pack(lo))
        return packed[pi]

    in_maps = []
    for core in range(NCORES):
        plan, i_list = _PLANS[core]
        fh = np.stack([fh_tiles[i] for i in i_list])
        fl = np.stack([fl_tiles[i] for i in i_list])
        wh = np.empty((_PER, P, KS * DA), dtype=BF16)
        wl = np.empty((_PER, P, KS * DA), dtype=BF16)
        for j, (pi, c) in enumerate(_CHUNKS[core * _PER : (core + 1) * _PER]):
            ph, pl = pair_chunks(pi)
            wh[j] = ph[c]
            wl[j] = pl[c]
        in_maps.append({"f_hi": fh, "f_lo": fl, "w_hi": wh, "w_lo": wl})
    return in_maps


def _assemble(results, b):
    out = np.zeros((B, L, DA), dtype=np.float32)
    for core in range(NCORES):
        plan, _ = _PLANS[core]
        o = np.asarray(results[core]["out"], dtype=np.float32)
        for seg_idx, (l, _i, _islot, _c0, _c1) in enumerate(plan):
            out[:, l, :] += o[seg_idx]
    out += np.asarray(b, dtype=np.float32)[None, :, :]
    return out


def _run_all(in_maps):
    """Dispatch the 8 per-core programs concurrently (thread per core)."""
    import concurrent.futures as cf

    import jax

    from concourse import bass2jax

    devices = jax.devices()[:NCORES]
    ncs = [_build_program(c) for c in range(NCORES)]

    def one(c):
        with jax.default_device(devices[c]):
            return bass2jax.run_bass_via_pjrt(ncs[c], [in_maps[c]], n_cores=1)[0]

    with cf.ThreadPoolExecutor(max_workers=NCORES) as ex:
        results = list(ex.map(one, range(NCORES)))
    return results


def _run_all_retry(in_maps, attempts=3):
    last = None
    for a in range(attempts):
        try:
            return _run_all(in_maps)
        except Exception as e:  # transient NRT_EXEC_UNIT_UNRECOVERABLE seen
            last = e
            print(f"kernel run attempt {a} failed ({e}); retrying")
    raise last


def run(inputs: dict, trace: bool = False, tmpdir: str | None = None):
    Ws = [np.asarray(inputs[f"W_{l}"], dtype=np.float32) for l in range(L)]
    in_maps = _prep_inputs(inputs["features"], Ws)

    if not trace:
        results = _run_all_retry(in_maps)
        return _assemble(results, inputs["b"]), None

    # tracing: wrap execution with the axon NTFF hook, then convert each
    # captured NTFF (one per core executable) to json via neuron-profile.
    import glob
    import json
    import re
    import subprocess
    import tempfile
    from dataclasses import dataclass

    from antenv.axon_hooks import get_axon_ntff_profile_hook

    hook = get_axon_ntff_profile_hook()
    neff_dir = tmpdir or tempfile.mkdtemp()
    with hook(neff_dir, None):
        results = _run_all(in_maps)
    out = _assemble(results, inputs["b"])

    times = []
    for ntff in sorted(glob.glob(neff_dir + "/*_body*.ntff")):
        m = re.search(r"(executable\d+)", ntff)
        neffs = glob.glob(neff_dir + f"/*{m.group(1)}.neff") if m else []
        if not neffs:
            continue
        jf = ntff + ".json"
        try:
            subprocess.check_call(
                [
                    "neuron-profile", "view", "--ignore-nc-buf-usage",
                    "-s", ntff, "-n", neffs[0],
                    "--output-format=json", f"--output-file={jf}",
                ],
                stdout=subprocess.DEVNULL, stderr=subprocess.DEVNULL,
            )
            with open(jf) as f:
                summ = json.load(f)["summary"][0]
            times.append((summ["total_time"] * 1e9, summ.get("nc_idx"), jf))
        except Exception as e:
            print("ntff convert failed:", ntff, e)
    times.sort(reverse=True)
    for t, nc_idx, jf in times:
        print(f"  core nc_idx={nc_idx}: {t:.0f} ns  ({jf})")

    @dataclass
    class R:
        exec_time_ns: int | None
        mean_exec_time_ns: float | None
        instructions_and_trace = None
        profile_json = None

    res = R(
        exec_time_ns=int(times[0][0]) if times else None,
        mean_exec_time_ns=(sum(t for t, _, _ in times) / len(times)) if times else None,
    )
    return out, res


def kernel(**inputs) -> np.ndarray:
    out, _ = run(inputs)
    return out
